# revision 8
# baseline (speedup 1.0000x reference)
"""NCC (3D local normalized cross-correlation) loss kernel for 8 Trainium2 cores.

Reference computes, for I, J of shape (2, 1, 160, 192, 224) fp32:
    5 box sums (7x7x7, zero-padded "same") of {I, J, I*I, J*J, I*J},
    cc = cross^2 / (I_var * J_var + eps),  result = -mean(cc).

Sharding: batch(2) x depth(4 slabs of 40) -> 8 cores. Each core gets
zero-padded depth slabs [46, 192, 224] and returns per-partition partial
sums of cc; the host reduces and takes the mean.

Per-core pipeline (one traced SPMD program):
  - depth slices streamed in pairs; per h-tile (rows 0..101 / 90..191):
    products I^2, J^2 (ScalarE Square) and I*J (VectorE),
    H-axis 7-tap box sum as a banded fp32r matmul on TensorE (band
    matrices passed in as constants),
    PSUM->SBUF evacuation casting to bf16,
    W-axis 7-tap box sum as a shifted-add tree on VectorE (padded tiles),
  - depth-axis 7-tap box sum as a hierarchical pair-sum tree,
  - cc combine + running reduction via tensor_tensor_reduce.
"""

import numpy as np

N_CORES = 8
B, D, H, W = 2, 160, 192, 224
WIN, PAD = 7, 3
WIN3 = float(WIN**3)
DSH = D // 4            # 40 output depths per core
SLAB = DSH + 2 * PAD    # 46 input slices per core
HT_ROWS = 102           # input h rows per h-tile
HT_OUT = 96             # output h' rows per h-tile
HT_R0 = (0, 90)
WPAD = W + 2 * PAD      # 230
NPAIRS = SLAB // 2      # 23
import os as _os
_NPAIRS_RUN = int(_os.environ.get('NCC_PAIRS', NPAIRS))
_NCORES_RUN = int(_os.environ.get('NCC_CORES', N_CORES))
_STAGE = int(_os.environ.get('NCC_STAGE', 4))

_CACHE = {}


def _build_bands() -> np.ndarray:
    bh = np.zeros((2, HT_ROWS, HT_OUT), np.float32)
    for ht in range(2):
        r0 = HT_R0[ht]
        for m in range(HT_OUT):
            hp = ht * HT_OUT + m          # output row h'
            for k in range(HT_ROWS):
                if abs((r0 + k) - hp) <= PAD:
                    bh[ht, k, m] = 1.0
    return bh


def _build_program():
    import concourse.bacc as bacc
    import concourse.mybir as mybir
    from concourse import tile

    f32 = mybir.dt.float32
    f32r = mybir.dt.float32r
    bf16 = mybir.dt.bfloat16
    Alu = mybir.AluOpType
    Act = mybir.ActivationFunctionType

    nc = bacc.Bacc("TRN2", target_bir_lowering=False, debug=False)
    dI = nc.dram_tensor("I", (SLAB, H, W), f32r, kind="ExternalInput").ap()
    dJ = nc.dram_tensor("J", (SLAB, H, W), f32r, kind="ExternalInput").ap()
    dBH = nc.dram_tensor("BH", (2, HT_ROWS, HT_OUT), f32r, kind="ExternalInput").ap()
    dOUT = nc.dram_tensor("PART", (HT_OUT, 1), f32, kind="ExternalOutput").ap()

    with tile.TileContext(nc) as tc:
        with (
            tc.tile_pool(name="consts", bufs=1) as cpool,
            tc.tile_pool(name="io", bufs=3) as iopool,
            tc.tile_pool(name="work", bufs=2) as wpool,
            tc.tile_pool(name="s7ring", bufs=5) as s7pool,
            tc.tile_pool(name="psring", bufs=4) as pspool,
            tc.tile_pool(name="psum", bufs=6, space="PSUM") as psumpool,
        ):
            bh = cpool.tile([HT_ROWS, 2, HT_OUT], f32r)
            nc.sync.dma_start(out=bh[:], in_=dBH.rearrange("t k m -> k t m"))
            part = cpool.tile([HT_OUT, 1], f32)
            strip = cpool.tile([HT_OUT, max(_NPAIRS_RUN - 3, 1)], f32)
            nc.vector.memset(strip[:], 0.0)
            trash = cpool.tile([HT_OUT, 2, 2, W], f32)
            ccv = cpool.tile([HT_OUT, 2, 2, W], f32)

            s7ring = []
            psring = []
            for p in range(_NPAIRS_RUN):
                s7 = s7pool.tile([HT_OUT, 5, 2, 2, W], bf16, tag="s7")
                for ht in range(2):
                    r0 = HT_R0[ht]
                    tIa = iopool.tile([HT_ROWS, 2, WPAD], f32r, tag="tI")
                    tJa = iopool.tile([HT_ROWS, 2, WPAD], f32r, tag="tJ")
                    for t_, d_ in ((tIa, dI), (tJa, dJ)):
                        nc.gpsimd.memset(t_[:, :, 0:PAD].bitcast(f32), 0.0)
                        nc.gpsimd.memset(t_[:, :, PAD + W : WPAD].bitcast(f32), 0.0)
                        nc.sync.dma_start(
                            out=t_[:, :, PAD : PAD + W],
                            in_=d_[2 * p : 2 * p + 2, r0 : r0 + HT_ROWS, :].rearrange(
                                "d h w -> h d w"
                            ),
                        )
                    prod = wpool.tile([HT_ROWS, 3, 2, WPAD], f32r, tag="prod")
                    nc.scalar.activation(prod[:, 0], tIa[:], Act.Square)
                    nc.scalar.activation(prod[:, 1], tJa[:], Act.Square)
                    nc.vector.tensor_mul(prod[:, 2], tIa[:], tJa[:])

                    ysb = wpool.tile([HT_OUT, 5, 2, WPAD], bf16, tag="ysb")
                    chans = [tIa[:], tJa[:], prod[:, 0], prod[:, 1], prod[:, 2]]
                    for c5 in range(5):
                        ps = psumpool.tile([HT_OUT, 2, WPAD], f32, tag="ypsum")
                        nc.tensor.matmul(
                            ps[:],
                            bh[:, ht, :],
                            chans[c5],
                            start=True,
                            stop=True,
                        )
                        if c5 % 2 == 0:
                            nc.scalar.copy(ysb[:, c5], ps[:])
                        else:
                            nc.vector.tensor_copy(out=ysb[:, c5], in_=ps[:])

                    # W-axis 7-tap tree on padded tiles: s7[i] = sum y[i..i+6]
                    s2 = wpool.tile([HT_OUT, 5, 2, 229], bf16, tag="s2")
                    s4 = wpool.tile([HT_OUT, 5, 2, 227], bf16, tag="s4")
                    s6 = wpool.tile([HT_OUT, 5, 2, 225], bf16, tag="s6")
                    nc.vector.tensor_add(s2[:], ysb[:, :, :, 0:229], ysb[:, :, :, 1:230])
                    nc.vector.tensor_add(s4[:], s2[:, :, :, 0:227], s2[:, :, :, 2:229])
                    nc.vector.tensor_add(s6[:], s4[:, :, :, 0:225], s2[:, :, :, 4:229])
                    nc.vector.tensor_add(
                        s7[:, :, ht], s6[:, :, :, 0:224], ysb[:, :, :, 6:230]
                    )
                s7ring.append(s7)
                pairsum = pspool.tile([HT_OUT, 5, 2, W], bf16, tag="ps")
                nc.vector.tensor_add(pairsum[:], s7[:, :, :, 0], s7[:, :, :, 1])
                psring.append(pairsum)

                if p < 3:
                    continue
                # Output depths od0=2p-6, od0+1; window od..od+6 over slab slices:
                # core = slices 2p-5..2p = s7[2p-5] + pair[p-2] + pair[p-1] + s7[2p]
                c1t = wpool.tile([HT_OUT, 5, 2, W], bf16, tag="c1t")
                core = wpool.tile([HT_OUT, 5, 2, W], bf16, tag="core")
                odb = wpool.tile([HT_OUT, 5, 2, 2, W], bf16, tag="odb")
                nc.vector.tensor_add(
                    c1t[:], s7ring[p - 3][:, :, :, 1], s7ring[p][:, :, :, 0]
                )
                nc.vector.tensor_add(core[:], psring[p - 2][:], psring[p - 1][:])
                nc.vector.tensor_add(core[:], core[:], c1t[:])
                nc.vector.tensor_add(
                    odb[:, :, :, 0], core[:], s7ring[p - 3][:, :, :, 0]
                )
                nc.vector.tensor_add(
                    odb[:, :, :, 1], core[:], s7ring[p][:, :, :, 1]
                )
                if _STAGE < 2:
                    continue

                # cc on [96, 2ht, 2od, 224]; channels: 0=Isum 1=Jsum 2=I2 3=J2 4=IJ
                a, b_, pp, qq, cij = (odb[:, c] for c in range(5))
                tab = wpool.tile([HT_OUT, 2, 2, W], bf16, tag="tab")
                crs = wpool.tile([HT_OUT, 2, 2, W], bf16, tag="crs")
                a2 = wpool.tile([HT_OUT, 2, 2, W], bf16, tag="a2")
                b2 = wpool.tile([HT_OUT, 2, 2, W], bf16, tag="b2")
                iv = wpool.tile([HT_OUT, 2, 2, W], bf16, tag="iv")
                jv = wpool.tile([HT_OUT, 2, 2, W], bf16, tag="jv")
                den = wpool.tile([HT_OUT, 2, 2, W], f32, tag="den")
                rr = wpool.tile([HT_OUT, 2, 2, W], f32, tag="rr")
                c2 = wpool.tile([HT_OUT, 2, 2, W], f32, tag="c2")
                nc.vector.tensor_mul(tab[:], a, b_)
                nc.vector.scalar_tensor_tensor(
                    crs[:], tab[:], -1.0 / WIN3, cij, Alu.mult, Alu.add
                )
                nc.scalar.activation(a2[:], a, Act.Square)
                nc.scalar.activation(b2[:], b_, Act.Square)
                nc.vector.scalar_tensor_tensor(
                    iv[:], a2[:], -1.0 / WIN3, pp, Alu.mult, Alu.add
                )
                nc.vector.scalar_tensor_tensor(
                    jv[:], b2[:], -1.0 / WIN3, qq, Alu.mult, Alu.add
                )
                nc.vector.tensor_mul(den[:], iv[:], jv[:])
                if _STAGE < 3:
                    continue
                nc.vector.reciprocal_approx_fast(
                    out=rr[:].rearrange("p a b w -> p (a b w)"),
                    in_=den[:].rearrange("p a b w -> p (a b w)"),
                )
                nc.scalar.activation(c2[:], crs[:], Act.Square)
                if _STAGE < 4:
                    continue
                nc.vector.tensor_mul(trash[:], c2[:], rr[:])
                nc.scalar.activation(
                    ccv[:],
                    trash[:],
                    Act.Copy,
                    accum_out=strip[:, p - 3 : p - 2],
                )

            nc.vector.reduce_sum(part[:], strip[:], axis=mybir.AxisListType.X)
            nc.sync.dma_start(out=dOUT, in_=part[:])

    nc.compile()
    return nc


def kernel(I: np.ndarray, J: np.ndarray) -> np.ndarray:
    from concourse.bass_utils import run_bass_kernel_spmd

    I = np.ascontiguousarray(np.asarray(I, dtype=np.float32))
    J = np.ascontiguousarray(np.asarray(J, dtype=np.float32))
    assert I.shape == (B, 1, D, H, W) and J.shape == I.shape

    if "nc" not in _CACHE:
        _CACHE["nc"] = _build_program()
    nc = _CACHE["nc"]
    bands = _build_bands()

    in_maps = []
    for c in range(N_CORES):
        bidx, s = divmod(c, 4)
        od0 = DSH * s
        m = {"BH": bands}
        for name, X in (("I", I), ("J", J)):
            sl = np.zeros((SLAB, H, W), np.float32)
            lo, hi = od0 - PAD, od0 + DSH + PAD
            slo, shi = max(lo, 0), min(hi, D)
            sl[slo - lo : shi - lo] = X[bidx, 0, slo:shi]
            m[name] = sl
        in_maps.append(m)

    in_maps = in_maps[:_NCORES_RUN]
    res = run_bass_kernel_spmd(nc, in_maps, core_ids=list(range(_NCORES_RUN)))
    total = sum(float(r["PART"].astype(np.float64).sum()) for r in res.results)
    mean = total / float(B * D * H * W)
    return np.float32(-mean)
